# revision 11
# baseline (speedup 1.0000x reference)
"""CFConv (gnn message passing) Trainium2 kernel.

Math (per batch b):
    h      = gelu(edge_features @ W1 + b1)        [N, K, C]
    W      = gelu(h @ W2 + b2)                    [N, K, C]
    x_j    = x[b][E_idx[b]]                       [N, K, C]
    out    = sum_k x_j * W                        [N, C]

Sharding: 8 cores = 4 batches x 2 node-halves (2048 nodes / core,
M = 61440 edge rows / core).

Host prep per core (layout only — all FLOPs stay on device):
  - edgeT [300, M] bf16: edge rows transposed so the E=300 contraction dim
    is the SBUF partition dim (contiguous per-partition DMA lines), split
    into E-chunks 128/128/44, cast to bf16 (the PE's fp32 matmul mode
    [fp32_mode=LOW_HIGH] is ~5x slower AND fp32 doubles the HBM traffic
    this memory-bound kernel is limited by).
  - xgT2 [128, 16*1920] f32: x[b][E_idx] gathered on host, transposed to
    channel-major, and group-PAIR stacked (rows 0:64 = even group's 64
    channels, 64:128 = odd group's) so every DVE/ACT op runs at the full
    128 partitions.
  - w2dup/b1dup/b2dup duplicated across both partition halves.

Device pipeline per pair of 1920-row groups (16 pairs of 2x64 nodes):
  mm1: two 3-chunk accumulating bf16 matmul chains (W1 stationary) into
  the two partition halves of one PSUM bank (chain B's start=True only
  clears has_written bits; chain A's finished data is untouched) ->
  gelu(+b1) [128,480] on ScalarE -> bf16 h -> mm2 (W2 stationary,
  row+col tile_position for the upper half) -> gelu(+b2) -> filter wT
  [128, 1920] f32 -> DVE multiply with the streamed x_j^T -> DVE
  groupwise reduce over K=30 -> [128, 64] -> DMA to a channel-major
  output staging tensor (host un-transposes 0.5MB at the end).
"""

import os
import sys

import numpy as np

sys.path.insert(0, "/opt/trn_rl_repo")

import ml_dtypes

import concourse.bacc as bacc
import concourse.tile as tile
from concourse import mybir
from concourse.bass_utils import run_bass_kernel_spmd

F32 = mybir.dt.float32
BF16 = mybir.dt.bfloat16
GELU = mybir.ActivationFunctionType.Gelu
BF = ml_dtypes.bfloat16

B, N, K, C, E = 4, 4096, 30, 64, 300
NCORES = 8
NPC = N // 2          # nodes per core
M = NPC * K           # edge rows per core = 61440
R = 1920              # rows per group = 64 nodes
NG = M // R           # 32 groups
NP_ = NG // 2         # 16 group pairs
NODESG = R // K       # 64 nodes per group
NSUB = 4
SUB = R // NSUB       # 480
EC = (128, 128, E - 256)  # E-chunk sizes

_CACHE = {}


def build_bass():
    nc = bacc.Bacc(
        "TRN2",
        target_bir_lowering=False,
        debug=False,
        enable_asserts=False,
        num_devices=NCORES,
    )
    e1 = nc.dram_tensor("e1", [128, M], BF16, kind="ExternalInput").ap()
    e2 = nc.dram_tensor("e2", [128, M], BF16, kind="ExternalInput").ap()
    e3p = nc.dram_tensor("e3p", [108, NP_ * R], BF16, kind="ExternalInput").ap()
    xgt = nc.dram_tensor("xgt", [128, NP_ * R], BF16, kind="ExternalInput").ap()
    w1 = nc.dram_tensor("w1", [E, C], BF16, kind="ExternalInput").ap()
    w1cd = nc.dram_tensor("w1cd", [108, C], BF16, kind="ExternalInput").ap()
    w2d = nc.dram_tensor("w2d", [128, C], BF16, kind="ExternalInput").ap()
    b1d = nc.dram_tensor("b1d", [128, 1], F32, kind="ExternalInput").ap()
    b2d = nc.dram_tensor("b2d", [128, 1], F32, kind="ExternalInput").ap()
    outT = nc.dram_tensor("outT", [128, NP_ * NODESG], F32, kind="ExternalOutput").ap()

    with tile.TileContext(nc) as tc:
        with (
            tc.tile_pool(name="const", bufs=1) as pconst,
            tc.tile_pool(name="edge", bufs=4) as pedge,
            tc.tile_pool(name="xjt", bufs=3) as pxjt,
            tc.tile_pool(name="hw", bufs=3) as phw,
            tc.tile_pool(name="mr", bufs=2) as pmr,
            tc.tile_pool(name="ot", bufs=2) as pot,
            tc.tile_pool(name="ps1", bufs=1, space="PSUM") as pps1,
            tc.tile_pool(name="ps2", bufs=1, space="PSUM") as pps2,
        ):
            w1a = pconst.tile([128, C], BF16, tag="w1a")
            nc.sync.dma_start(w1a[:], w1[0:128, :])
            w1b = pconst.tile([128, C], BF16, tag="w1b")
            nc.sync.dma_start(w1b[:], w1[128:256, :])
            w1cs = pconst.tile([108, C], BF16, tag="w1cs")
            nc.sync.dma_start(w1cs[:], w1cd)
            w2s = pconst.tile([128, C], BF16, tag="w2s")
            nc.sync.dma_start(w2s[:], w2d)
            b1s = pconst.tile([128, 1], F32, tag="b1s")
            nc.sync.dma_start(b1s[:], b1d)
            b2s = pconst.tile([128, 1], F32, tag="b2s")
            nc.sync.dma_start(b2s[:], b2d)

            for u in range(NP_):
                c0 = 2 * u * R  # columns of the pair (two adjacent groups)
                t1 = pedge.tile([128, 2 * R], BF16, tag="t1")
                nc.sync.dma_start(t1[:], e1[:, c0 : c0 + 2 * R])
                t2 = pedge.tile([128, 2 * R], BF16, tag="t2")
                nc.sync.dma_start(t2[:], e2[:, c0 : c0 + 2 * R])
                t3 = pedge.tile([108, R], BF16, tag="t3")
                nc.sync.dma_start(t3[:], e3p[:, u * R : (u + 1) * R])
                xjt = pxjt.tile([128, R], BF16)
                nc.sync.dma_start(xjt[:], xgt[:, u * R : (u + 1) * R])

                h2 = phw.tile([128, R], BF16, tag="h2")
                wt2 = phw.tile([128, R], BF16, tag="wt2")
                # mm1, weight-stationary ("chunk-outer") order: each W1
                # chunk is loaded once per column-group chain and streams
                # all 4 subtile banks. PE MATMULs execute in strict FIFO
                # emission order, so within each bank the accumulation
                # chain A fully precedes chain B's start=True (which
                # clears only has_written bits; A's finished data stays).
                ps1s = [pps1.tile([128, SUB], F32, tag=f"ps1_{t}", name=f"ps1_{t}") for t in range(NSUB)]
                for cg in (0, 1):
                    po = slice(0, C) if cg == 0 else slice(C, 128)
                    base = cg * R
                    rp = slice(0, 44) if cg == 0 else slice(64, 108)
                    chunks = (
                        (w1a[:], t1, base, (0, 0) if cg == 0 else (0, 64)),
                        (w1b[:], t2, base, (0, 0) if cg == 0 else (0, 64)),
                        (w1cs[rp, :], t3, 0, (0, 0) if cg == 0 else (64, 64)),
                    )
                    for ci, (wch, ech, boff, tp) in enumerate(chunks):
                        for t in range(NSUB):
                            s = slice(boff + t * SUB, boff + (t + 1) * SUB)
                            rhs = ech[rp, s] if ci == 2 else ech[:, s]
                            nc.tensor.matmul(
                                ps1s[t][po, :],
                                wch,
                                rhs,
                                start=(ci == 0),
                                stop=(ci == 2),
                                tile_position=tp,
                                skip_group_check=True,
                            )
                ps2s = [pps2.tile([128, SUB], F32, tag=f"ps2_{t}", name=f"ps2_{t}") for t in range(NSUB)]
                for t in range(NSUB):
                    s = slice(t * SUB, (t + 1) * SUB)
                    nc.scalar.activation(h2[:, s], ps1s[t][:], GELU, bias=b1s[:])
                for cg in (0, 1):
                    po = slice(0, C) if cg == 0 else slice(C, 128)
                    tp = None if cg == 0 else (64, 64)
                    for t in range(NSUB):
                        s = slice(t * SUB, (t + 1) * SUB)
                        nc.tensor.matmul(
                            ps2s[t][po, :],
                            w2s[po, :],
                            h2[po, s],
                            start=True,
                            stop=True,
                            tile_position=tp,
                            skip_group_check=True,
                        )
                for t in range(NSUB):
                    s = slice(t * SUB, (t + 1) * SUB)
                    nc.scalar.activation(wt2[:, s], ps2s[t][:], GELU, bias=b2s[:])

                mr2 = pmr.tile([128, R], BF16)
                nc.vector.tensor_mul(mr2[:], wt2[:], xjt[:])
                ot2 = pot.tile([128, NODESG], F32)
                nc.vector.tensor_reduce(
                    ot2[:],
                    mr2[:].rearrange("p (n k) -> p n k", k=K),
                    axis=mybir.AxisListType.X,
                    op=mybir.AluOpType.add,
                )
                nc.sync.dma_start(outT[:, u * NODESG : (u + 1) * NODESG], ot2[:])

    nc.compile()
    return nc


def prep_in_maps(x, edge_features, E_idx, W1, b1, W2, b2):
    x = np.asarray(x, dtype=np.float32)
    edge_features = np.asarray(edge_features, dtype=np.float32)
    E_idx = np.asarray(E_idx)
    W1 = np.asarray(W1, dtype=np.float32)
    b1 = np.asarray(b1, dtype=np.float32)
    W2 = np.asarray(W2, dtype=np.float32)
    b2 = np.asarray(b2, dtype=np.float32)

    shared = {
        "w1": np.ascontiguousarray(W1).astype(BF),
        "w2d": np.ascontiguousarray(np.concatenate([W2, W2], axis=0)).astype(BF),
        "w1cd": np.concatenate(
            [
                W1[256:E],
                np.zeros((20, C), np.float32),
                W1[256:E],
            ],
            axis=0,
        ).astype(BF),
        "b1d": np.tile(b1.reshape(C, 1), (2, 1)).astype(np.float32),
        "b2d": np.tile(b2.reshape(C, 1), (2, 1)).astype(np.float32),
    }
    in_maps = []
    for c in range(NCORES):
        b = c // 2
        n0 = (c % 2) * NPC
        ef = edge_features[b, n0 : n0 + NPC].reshape(M, E)
        edgeT = np.ascontiguousarray(ef.T.astype(BF))
        idx = np.ascontiguousarray(E_idx[b, n0 : n0 + NPC]).reshape(M).astype(np.int64)
        xg = x[b][idx]  # [M, C] f32 host gather
        xjt = np.ascontiguousarray(xg.T)  # [C, M]
        xx = xjt.reshape(C, NP_, 2, R)
        xgt = np.ascontiguousarray(
            np.concatenate([xx[:, :, 0, :], xx[:, :, 1, :]], axis=0).reshape(
                128, NP_ * R
            )
        )
        et3 = edgeT[256:E].reshape(E - 256, NP_, 2, R)
        e3p = np.zeros((108, NP_ * R), dtype=BF)
        e3p.reshape(108, NP_, R)[0 : E - 256] = et3[:, :, 0, :]
        e3p.reshape(108, NP_, R)[64 : 64 + E - 256] = et3[:, :, 1, :]
        in_maps.append(
            dict(
                shared,
                e1=edgeT[0:128],
                e2=edgeT[128:256],
                e3p=e3p,
                xgt=xgt.astype(BF),
            )
        )
    return in_maps


def unshard_out(results):
    out = np.empty((B, N, C), dtype=np.float32)
    for c in range(NCORES):
        b = c // 2
        n0 = (c % 2) * NPC
        o = results[c]["outT"].reshape(128, NP_, NODESG)
        loc = np.empty((NP_, 2, NODESG, C), dtype=np.float32)
        loc[:, 0] = o[0:C].transpose(1, 2, 0)
        loc[:, 1] = o[C:128].transpose(1, 2, 0)
        out[b, n0 : n0 + NPC] = loc.reshape(NPC, C)
    return out


def run(in_maps, trace=False):
    if "nc" not in _CACHE:
        _CACHE["nc"] = build_bass()
    nc = _CACHE["nc"]
    kw = {}
    if trace:
        kw["trace"] = True
    res = run_bass_kernel_spmd(nc, in_maps, core_ids=list(range(NCORES)), **kw)
    return res


def kernel(x, edge_features, E_idx, W1, b1, W2, b2):
    in_maps = prep_in_maps(x, edge_features, E_idx, W1, b1, W2, b2)
    res = run(in_maps, trace=bool(os.environ.get("CFCONV_TRACE")))
    if getattr(res, "exec_time_ns", None) is not None:
        print(f"HW exec time: {res.exec_time_ns} ns")
    return unshard_out(res.results)


# revision 12
# speedup vs baseline: 1.2116x; 1.2116x over previous
"""CFConv (gnn message passing) Trainium2 kernel.

Math (per batch b):
    h      = gelu(edge_features @ W1 + b1)        [N, K, C]
    W      = gelu(h @ W2 + b2)                    [N, K, C]
    x_j    = x[b][E_idx[b]]                       [N, K, C]
    out    = sum_k x_j * W                        [N, C]

Sharding: 8 cores = 4 batches x 2 node-halves (2048 nodes / core,
M = 61440 edge rows / core).

Host prep per core (layout only — all FLOPs stay on device):
  - edgeT [300, M] bf16: edge rows transposed so the E=300 contraction dim
    is the SBUF partition dim (contiguous per-partition DMA lines), split
    into E-chunks 128/128/44, cast to bf16 (the PE's fp32 matmul mode
    [fp32_mode=LOW_HIGH] is ~5x slower AND fp32 doubles the HBM traffic
    this memory-bound kernel is limited by).
  - xgT2 [128, 16*1920] f32: x[b][E_idx] gathered on host, transposed to
    channel-major, and group-PAIR stacked (rows 0:64 = even group's 64
    channels, 64:128 = odd group's) so every DVE/ACT op runs at the full
    128 partitions.
  - w2dup/b1dup/b2dup duplicated across both partition halves.

Device pipeline per pair of 1920-row groups (16 pairs of 2x64 nodes):
  mm1: two 3-chunk accumulating bf16 matmul chains (W1 stationary) into
  the two partition halves of one PSUM bank (chain B's start=True only
  clears has_written bits; chain A's finished data is untouched) ->
  gelu(+b1) [128,480] on ScalarE -> bf16 h -> mm2 (W2 stationary,
  row+col tile_position for the upper half) -> gelu(+b2) -> filter wT
  [128, 1920] f32 -> DVE multiply with the streamed x_j^T -> DVE
  groupwise reduce over K=30 -> [128, 64] -> DMA to a channel-major
  output staging tensor (host un-transposes 0.5MB at the end).
"""

import os
import sys

import numpy as np

sys.path.insert(0, "/opt/trn_rl_repo")

import ml_dtypes

import concourse.bacc as bacc
import concourse.tile as tile
from concourse import mybir
from concourse.bass_utils import run_bass_kernel_spmd

F32 = mybir.dt.float32
BF16 = mybir.dt.bfloat16
GELU = mybir.ActivationFunctionType.Gelu
BF = ml_dtypes.bfloat16

B, N, K, C, E = 4, 4096, 30, 64, 300
NCORES = 8
NPC = N // 2          # nodes per core
M = NPC * K           # edge rows per core = 61440
R = 1920              # rows per group = 64 nodes
NG = M // R           # 32 groups
NP_ = NG // 2         # 16 group pairs
NODESG = R // K       # 64 nodes per group
NSUB = 4
SUB = R // NSUB       # 480
EC = (128, 128, E - 256)  # E-chunk sizes

_CACHE = {}


def build_bass():
    nc = bacc.Bacc(
        "TRN2",
        target_bir_lowering=False,
        debug=False,
        enable_asserts=False,
        num_devices=NCORES,
    )
    e1 = nc.dram_tensor("e1", [128, M], BF16, kind="ExternalInput").ap()
    e2 = nc.dram_tensor("e2", [128, M], BF16, kind="ExternalInput").ap()
    e3p = nc.dram_tensor("e3p", [108, NP_ * R], BF16, kind="ExternalInput").ap()
    xgt = nc.dram_tensor("xgt", [128, NP_ * R], BF16, kind="ExternalInput").ap()
    w1 = nc.dram_tensor("w1", [E, C], BF16, kind="ExternalInput").ap()
    w1cd = nc.dram_tensor("w1cd", [108, C], BF16, kind="ExternalInput").ap()
    w2d = nc.dram_tensor("w2d", [128, C], BF16, kind="ExternalInput").ap()
    b1d = nc.dram_tensor("b1d", [128, 1], F32, kind="ExternalInput").ap()
    b2d = nc.dram_tensor("b2d", [128, 1], F32, kind="ExternalInput").ap()
    outT = nc.dram_tensor("outT", [128, NP_ * NODESG], F32, kind="ExternalOutput").ap()

    with tile.TileContext(nc) as tc:
        with (
            tc.tile_pool(name="const", bufs=1) as pconst,
            tc.tile_pool(name="edge", bufs=3) as pedge,
            tc.tile_pool(name="xjt", bufs=2) as pxjt,
            tc.tile_pool(name="hw", bufs=2) as phw,
            tc.tile_pool(name="mr", bufs=2) as pmr,
            tc.tile_pool(name="ot", bufs=2) as pot,
            tc.tile_pool(name="ps1", bufs=1, space="PSUM") as pps1,
            tc.tile_pool(name="ps2", bufs=1, space="PSUM") as pps2,
        ):
            w1a = pconst.tile([128, C], BF16, tag="w1a")
            nc.sync.dma_start(w1a[:], w1[0:128, :])
            w1b = pconst.tile([128, C], BF16, tag="w1b")
            nc.sync.dma_start(w1b[:], w1[128:256, :])
            w1cs = pconst.tile([108, C], BF16, tag="w1cs")
            nc.sync.dma_start(w1cs[:], w1cd)
            w2s = pconst.tile([128, C], BF16, tag="w2s")
            nc.sync.dma_start(w2s[:], w2d)
            b1s = pconst.tile([128, 1], F32, tag="b1s")
            nc.sync.dma_start(b1s[:], b1d)
            b2s = pconst.tile([128, 1], F32, tag="b2s")
            nc.sync.dma_start(b2s[:], b2d)

            for u in range(NP_):
                c0 = 2 * u * R  # columns of the pair (two adjacent groups)
                t1 = pedge.tile([128, 2 * R], BF16, tag="t1")
                nc.sync.dma_start(t1[:], e1[:, c0 : c0 + 2 * R])
                t2 = pedge.tile([128, 2 * R], BF16, tag="t2")
                nc.sync.dma_start(t2[:], e2[:, c0 : c0 + 2 * R])
                t3 = pedge.tile([108, R], BF16, tag="t3")
                nc.sync.dma_start(t3[:], e3p[:, u * R : (u + 1) * R])
                xjt = pxjt.tile([128, R], BF16)
                nc.sync.dma_start(xjt[:], xgt[:, u * R : (u + 1) * R])

                h2 = phw.tile([128, R], BF16, tag="h2")
                wt2 = phw.tile([128, R], BF16, tag="wt2")
                # mm1, weight-stationary ("chunk-outer") order: each W1
                # chunk is loaded once per column-group chain and streams
                # all 4 subtile banks. PE MATMULs execute in strict FIFO
                # emission order, so within each bank the accumulation
                # chain A fully precedes chain B's start=True (which
                # clears only has_written bits; A's finished data stays).
                ps1s = [pps1.tile([128, SUB], F32, tag=f"ps1_{t}", name=f"ps1_{t}") for t in range(NSUB)]
                for cg in (0, 1):
                    po = slice(0, C) if cg == 0 else slice(C, 128)
                    base = cg * R
                    rp = slice(0, 44) if cg == 0 else slice(64, 108)
                    chunks = (
                        (w1a[:], t1, base, (0, 0) if cg == 0 else (0, 64)),
                        (w1b[:], t2, base, (0, 0) if cg == 0 else (0, 64)),
                        (w1cs[rp, :], t3, 0, (0, 0) if cg == 0 else (64, 64)),
                    )
                    for ci, (wch, ech, boff, tp) in enumerate(chunks):
                        for t in range(NSUB):
                            s = slice(boff + t * SUB, boff + (t + 1) * SUB)
                            rhs = ech[rp, s] if ci == 2 else ech[:, s]
                            nc.tensor.matmul(
                                ps1s[t][po, :],
                                wch,
                                rhs,
                                start=(ci == 0),
                                stop=(ci == 2),
                                tile_position=tp,
                                skip_group_check=True,
                            )
                ps2s = [pps2.tile([128, SUB], F32, tag=f"ps2_{t}", name=f"ps2_{t}") for t in range(NSUB)]
                for t in range(NSUB):
                    s = slice(t * SUB, (t + 1) * SUB)
                    nc.scalar.activation(h2[:, s], ps1s[t][:], GELU, bias=b1s[:])
                for cg in (0, 1):
                    po = slice(0, C) if cg == 0 else slice(C, 128)
                    tp = None if cg == 0 else (64, 64)
                    for t in range(NSUB):
                        s = slice(t * SUB, (t + 1) * SUB)
                        nc.tensor.matmul(
                            ps2s[t][po, :],
                            w2s[po, :],
                            h2[po, s],
                            start=True,
                            stop=True,
                            tile_position=tp,
                            skip_group_check=True,
                        )
                for t in range(NSUB):
                    s = slice(t * SUB, (t + 1) * SUB)
                    nc.scalar.activation(wt2[:, s], ps2s[t][:], GELU, bias=b2s[:])

                mr2 = pmr.tile([128, R], BF16)
                nc.vector.tensor_mul(mr2[:], wt2[:], xjt[:])
                ot2 = pot.tile([128, NODESG], F32)
                nc.vector.tensor_reduce(
                    ot2[:],
                    mr2[:].rearrange("p (n k) -> p n k", k=K),
                    axis=mybir.AxisListType.X,
                    op=mybir.AluOpType.add,
                )
                nc.sync.dma_start(outT[:, u * NODESG : (u + 1) * NODESG], ot2[:])

    nc.compile()
    return nc


def prep_in_maps(x, edge_features, E_idx, W1, b1, W2, b2):
    x = np.asarray(x, dtype=np.float32)
    edge_features = np.asarray(edge_features, dtype=np.float32)
    E_idx = np.asarray(E_idx)
    W1 = np.asarray(W1, dtype=np.float32)
    b1 = np.asarray(b1, dtype=np.float32)
    W2 = np.asarray(W2, dtype=np.float32)
    b2 = np.asarray(b2, dtype=np.float32)

    shared = {
        "w1": np.ascontiguousarray(W1).astype(BF),
        "w2d": np.ascontiguousarray(np.concatenate([W2, W2], axis=0)).astype(BF),
        "w1cd": np.concatenate(
            [
                W1[256:E],
                np.zeros((20, C), np.float32),
                W1[256:E],
            ],
            axis=0,
        ).astype(BF),
        "b1d": np.tile(b1.reshape(C, 1), (2, 1)).astype(np.float32),
        "b2d": np.tile(b2.reshape(C, 1), (2, 1)).astype(np.float32),
    }
    in_maps = []
    for c in range(NCORES):
        b = c // 2
        n0 = (c % 2) * NPC
        ef = edge_features[b, n0 : n0 + NPC].reshape(M, E)
        edgeT = np.ascontiguousarray(ef.T.astype(BF))
        idx = np.ascontiguousarray(E_idx[b, n0 : n0 + NPC]).reshape(M).astype(np.int64)
        xg = x[b][idx]  # [M, C] f32 host gather
        xjt = np.ascontiguousarray(xg.T)  # [C, M]
        xx = xjt.reshape(C, NP_, 2, R)
        xgt = np.ascontiguousarray(
            np.concatenate([xx[:, :, 0, :], xx[:, :, 1, :]], axis=0).reshape(
                128, NP_ * R
            )
        )
        et3 = edgeT[256:E].reshape(E - 256, NP_, 2, R)
        e3p = np.zeros((108, NP_ * R), dtype=BF)
        e3p.reshape(108, NP_, R)[0 : E - 256] = et3[:, :, 0, :]
        e3p.reshape(108, NP_, R)[64 : 64 + E - 256] = et3[:, :, 1, :]
        in_maps.append(
            dict(
                shared,
                e1=edgeT[0:128],
                e2=edgeT[128:256],
                e3p=e3p,
                xgt=xgt.astype(BF),
            )
        )
    return in_maps


def unshard_out(results):
    out = np.empty((B, N, C), dtype=np.float32)
    for c in range(NCORES):
        b = c // 2
        n0 = (c % 2) * NPC
        o = results[c]["outT"].reshape(128, NP_, NODESG)
        loc = np.empty((NP_, 2, NODESG, C), dtype=np.float32)
        loc[:, 0] = o[0:C].transpose(1, 2, 0)
        loc[:, 1] = o[C:128].transpose(1, 2, 0)
        out[b, n0 : n0 + NPC] = loc.reshape(NPC, C)
    return out


def run(in_maps, trace=False):
    if "nc" not in _CACHE:
        _CACHE["nc"] = build_bass()
    nc = _CACHE["nc"]
    kw = {}
    if trace:
        kw["trace"] = True
    res = run_bass_kernel_spmd(nc, in_maps, core_ids=list(range(NCORES)), **kw)
    return res


def kernel(x, edge_features, E_idx, W1, b1, W2, b2):
    in_maps = prep_in_maps(x, edge_features, E_idx, W1, b1, W2, b2)
    res = run(in_maps, trace=bool(os.environ.get("CFCONV_TRACE")))
    if getattr(res, "exec_time_ns", None) is not None:
        print(f"HW exec time: {res.exec_time_ns} ns")
    return unshard_out(res.results)
